# revision 16
# baseline (speedup 1.0000x reference)
"""Trainium2 Bass kernel for Bahdanau-style additive attention.

Computation (per batch row b):
    q_proj = query[b] @ Ws + bs                       # [U]
    v_proj = values[b] @ Wt + bt                      # [S, U]
    score  = tanh(q_proj + v_proj) @ Wa + ba          # [S]
    w      = softmax(score)                           # [S]
    att    = sum_s w[s] * values[b, s]                # [D]
Returns (att [B, D], w [B, S, 1]).

Sharding: data-parallel over batch across 8 NeuronCores (4 rows each);
dense weights replicated.  No collectives needed; results are gathered
on the host.

Notes on the on-device dataflow (per core, per batch row):
  - values arrive in natural [s, d] layout; each [128, 128] block is
    transposed on the PE (transpose-mode matmul) to get the d-major
    layout required as the stationary operand of the projection matmul.
  - projection psum accumulates  valT.T @ Wt  over 4 d-chunks plus a
    K=1 rank-1 matmul that adds (query@Ws + bs + bt) broadcast along
    partitions.
  - tanh on the scalar engine (PSUM -> SBUF), then one DVE
    tensor_tensor_reduce per s-tile computes score = sum_u tanh * Wa.
  - softmax skips the max-subtraction (scores are bounded by ||Wa||_1,
    softmax is shift-invariant; exp stays well inside fp32 range) so
    exp/sums can stream without a batch-global barrier.
  - attention accumulates  exp_col.T @ values_tile  into one psum bank
    over all 32 s-tiles; normalization by 1/sum(exp) happens on the
    [1, 512] result and on the transposed weight tile at the end.
  - matmul operands are viewed as float32r (full-rate fp32 streaming on
    TRN2; plain float32 matmuls run at quarter rate).
"""

import sys

for _p in ("/opt/trn_rl_repo", "/root/.axon_site/_ro/trn_rl_repo"):
    if _p not in sys.path:
        sys.path.append(_p)

from contextlib import ExitStack

import numpy as np

import concourse.bass as bass
import concourse.mybir as mybir
import concourse.tile as tile
from concourse import bacc
from concourse.masks import make_identity

dt = mybir.dt
F32 = dt.float32
F32R = dt.float32r
AF = mybir.ActivationFunctionType
ALU = mybir.AluOpType

B, S, D, U = 32, 4096, 512, 512
NCORES = 8
BSH = B // NCORES  # 4 batch rows per core
P = 128
DC = D // P        # 4 d-chunks
ST = S // P        # 32 s-tiles per batch row
GRP = 4            # s-tiles per group (group = 512 s rows = 1 DMA)
NG = ST // GRP     # 8 groups
ATT_LAG = 2        # groups of lag before emitting attention matmuls
MM_LAG = 2         # s-tiles of lag between transpose stage and projection stage


def _r(ap):
    """View an AP as float32r for PE ops (full-rate fp32 matmul)."""
    return ap.bitcast(F32R)


def ts(i, size):
    return slice(i * size, (i + 1) * size)


def build_nc():
    nc = bacc.Bacc("TRN2", target_bir_lowering=False, debug=False,
                   num_devices=NCORES)

    query = nc.declare_dram_parameter("query", [BSH, D], F32, isOutput=False).ap()
    values = nc.declare_dram_parameter("values", [BSH, S, D], F32, isOutput=False).ap()
    Ws = nc.declare_dram_parameter("Ws", [D, U], F32, isOutput=False).ap()
    bs_d = nc.declare_dram_parameter("bs", [U], F32, isOutput=False).ap()
    Wt = nc.declare_dram_parameter("Wt", [D, U], F32, isOutput=False).ap()
    bt_d = nc.declare_dram_parameter("bt", [U], F32, isOutput=False).ap()
    Wa = nc.declare_dram_parameter("Wa", [U, 1], F32, isOutput=False).ap()
    att_out = nc.declare_dram_parameter("attention", [BSH, D], F32, isOutput=True).ap()
    aw_out = nc.declare_dram_parameter("attention_weights", [BSH, S], F32,
                                       isOutput=True).ap()

    with tile.TileContext(nc) as tc:
        with ExitStack() as ctx:
            _build_program(ctx, tc, query, values, Ws, bs_d, Wt, bt_d, Wa,
                           att_out, aw_out)
    nc.compile()
    return nc


def _build_program(ctx, tc, query, values, Ws, bs_d, Wt, bt_d, Wa,
                   att_out, aw_out):
    nc = tc.nc

    const = ctx.enter_context(tc.tile_pool(name="const", bufs=1))
    vals_pool = ctx.enter_context(tc.tile_pool(name="vals", bufs=8))
    vt_pool = ctx.enter_context(tc.tile_pool(name="vt", bufs=5))
    tanh_pool = ctx.enter_context(tc.tile_pool(name="tanh", bufs=3))
    scr_pool = ctx.enter_context(tc.tile_pool(name="scr", bufs=2))
    small = ctx.enter_context(tc.tile_pool(name="small", bufs=2))
    ps_vproj = ctx.enter_context(tc.tile_pool(name="ps_vproj", bufs=3, space="PSUM"))
    ps_valt = ctx.enter_context(tc.tile_pool(name="ps_valt", bufs=2, space="PSUM"))
    ps_att = ctx.enter_context(tc.tile_pool(name="ps_att", bufs=2, space="PSUM"))
    ps_misc = ctx.enter_context(tc.tile_pool(name="ps_misc", bufs=1, space="PSUM"))

    # ---------------- constants ----------------
    ident_f = const.tile([P, P], F32)
    make_identity(nc, ident_f)
    ident = const.tile([P, P], F32R)
    nc.vector.tensor_copy(ident, ident_f)
    ones_f = const.tile([1, P], F32)
    nc.vector.memset(ones_f, 1.0)
    ones_row = const.tile([1, P], F32R)
    nc.vector.tensor_copy(ones_row, ones_f)
    ones_colf = const.tile([P, 1], F32)
    nc.vector.memset(ones_colf, 1.0)

    # query first (tiny, unblocks q-projection), then Ws, then Wt.
    query_sb = const.tile([BSH, D], F32R)
    nc.sync.dma_start(out=query_sb, in_=query.bitcast(F32R))
    ws_sb = const.tile([P, DC, U], F32R)
    nc.sync.dma_start(out=ws_sb, in_=Ws.bitcast(F32R).rearrange("(k p) u -> p k u", p=P))
    wt_sb = const.tile([P, DC, U], F32R)
    nc.sync.dma_start(out=wt_sb, in_=Wt.bitcast(F32R).rearrange("(k p) u -> p k u", p=P))

    # Wa broadcast along partitions: [128, 512], every partition = Wa[:, 0].
    wa_b = const.tile([P, U], F32)
    wa_flat = Wa.rearrange("u one -> (u one)")
    wa_bcast_in = bass.AP(tensor=wa_flat.tensor, offset=wa_flat.offset,
                          ap=[[0, P]] + list(wa_flat.ap))
    nc.sync.dma_start(out=wa_b, in_=wa_bcast_in)

    # bias rows on partition 0
    bs_row = const.tile([1, U], F32)
    nc.sync.dma_start(out=bs_row, in_=bs_d)
    bt_row = const.tile([1, U], F32)
    nc.sync.dma_start(out=bt_row, in_=bt_d)
    bst_row = const.tile([1, U], F32R)
    nc.vector.tensor_add(bst_row, bs_row, bt_row)

    # ---------------- q projection (once per core) ----------------
    qt_sb = const.tile([P, DC, BSH], F32R)  # query transposed, d-chunk k at [:, k, :]
    for k in range(DC):
        qt_ps = ps_valt.tile([P, BSH], F32R, tag="vt_ps", name=f"qt_ps{k}")
        nc.tensor.transpose(out=qt_ps, in_=query_sb[:, ts(k, P)],
                            identity=ident[0:BSH, 0:BSH])
        nc.vector.tensor_copy(qt_sb[:, k, :], qt_ps)

    qproj_ps = ps_vproj.tile([BSH, U], F32, tag="pj", name="qproj_ps")
    for k in range(DC):
        nc.tensor.matmul(qproj_ps, lhsT=qt_sb[:, k, :], rhs=ws_sb[:, k, :],
                         start=(k == 0), stop=False)
    # += (bs + bt) broadcast along partitions via K=1 rank-1 matmul
    nc.tensor.matmul(qproj_ps, lhsT=ones_row[:, 0:BSH], rhs=bst_row,
                     start=False, stop=True)
    qproj_sb = const.tile([BSH, U], F32R)
    nc.vector.tensor_copy(qproj_sb, qproj_ps)

    # per-batch bias rows staged on partition 0 (for the K=1 add matmul)
    qb_stage = []
    for b in range(BSH):
        qb = const.tile([1, U], F32R, name=f"qb_stage{b}")
        nc.sync.dma_start(out=qb, in_=qproj_sb[b:b + 1, :])
        qb_stage.append(qb)

    # ---------------- main pipeline ----------------
    # Emission is software-pipelined so no in-order engine queue convoys:
    #   - the DVE score-reduce for s-tile j is emitted after s-tile j+1's
    #     PE work (its tanh/mul chain lags the PE by ~2 ops),
    #   - exp for group g is emitted one group later,
    #   - attention matmuls lag 2 groups; the last 2 groups drain into the
    #     NEXT batch's loop (att_drain), so the PE never sits on the
    #     exp chain at a batch boundary.
    att_drain = []

    def s_loop(b, state):
        score_sb = small.tile([P, ST], F32, tag="score", name=f"score{b}")
        exp_f = small.tile([P, ST], F32, tag="expf", name=f"expf{b}")
        exp_t = small.tile([P, ST], F32R, tag="exp", name=f"exp{b}")
        colsums = small.tile([P, NG], F32, tag="colsums", name=f"colsums{b}")
        att_ps = ps_att.tile([1, U], F32, tag="att", name=f"att_ps{b}")
        vals_g = []
        prods = {}

        def att_group(g):
            for j in range(GRP):
                jj = g * GRP + j
                nc.tensor.matmul(att_ps, lhsT=exp_t[:, jj:jj + 1],
                                 rhs=vals_g[g][:, j, :],
                                 start=(jj == 0), stop=(jj == ST - 1))

        vts = {}

        def emit_T(g, j):
            """Transpose stage: PE block transposes + psum->sbuf copy."""
            jj = g * GRP + j
            nat = vals_g[g][:, j, :]
            vt_ps = ps_valt.tile([P, D], F32R, tag="vt_ps", name=f"vt_ps{b}_{jj}")
            for k in range(DC):
                nc.tensor.transpose(out=vt_ps[:, ts(k, P)], in_=nat[:, ts(k, P)],
                                    identity=ident)
            vt_sb = vt_pool.tile([P, D], F32R, tag="vt", name=f"vt_sb{b}_{jj}")
            if jj % 2 == 0:
                nc.vector.tensor_copy(vt_sb, vt_ps)
            else:
                nc.scalar.copy(vt_sb, vt_ps)
            vts[jj] = vt_sb

        def emit_MM(jj):
            """Projection stage: 4 accumulating matmuls + bias + tanh + Wa-mul."""
            vt_sb = vts.pop(jj)
            pj = ps_vproj.tile([P, U], F32, tag="pj", name=f"pj{b}_{jj}")
            for k in range(DC):
                nc.tensor.matmul(pj, lhsT=vt_sb[:, ts(k, P)], rhs=wt_sb[:, k, :],
                                 start=(k == 0), stop=False)
            nc.tensor.matmul(pj, lhsT=ones_row, rhs=qb_stage[b], start=False,
                             stop=True)
            th = tanh_pool.tile([P, U], F32, tag="th", name=f"th{b}_{jj}")
            nc.scalar.activation(out=th, in_=pj, func=AF.Tanh)
            prod = scr_pool.tile([P, U], F32, tag="prod", name=f"prod{b}_{jj}")
            nc.gpsimd.tensor_mul(prod, th, wa_b)
            prods[jj] = prod

        def emit_reduce(jj):
            nc.vector.reduce_sum(score_sb[:, jj:jj + 1], prods.pop(jj),
                                 axis=mybir.AxisListType.X)

        def emit_exp(g):
            nc.scalar.activation(out=exp_f[:, ts(g, GRP)],
                                 in_=score_sb[:, ts(g, GRP)],
                                 func=AF.Exp, accum_out=colsums[:, g:g + 1])
            nc.gpsimd.tensor_copy(exp_t[:, ts(g, GRP)], exp_f[:, ts(g, GRP)])

        for g in range(NG):
            vg = vals_pool.tile([P, GRP, D], F32R, tag="vals", name=f"vals{b}_{g}")
            vals_g.append(vg)
            nc.sync.dma_start(
                out=vg,
                in_=values.bitcast(F32R)[b, ts(g, GRP * P), :]
                .rearrange("(t p) d -> p t d", p=P))
            for j in range(GRP):
                jj = g * GRP + j
                emit_T(g, j)
                if jj >= MM_LAG:
                    emit_MM(jj - MM_LAG)
                if jj > MM_LAG:
                    emit_reduce(jj - MM_LAG - 1)
                if j == 3 and g >= 1:
                    emit_exp(g - 1)
                if j == 2:
                    if g >= 2:
                        att_group(g - 2)
                    elif att_drain:
                        att_drain.pop(0)()
        for jj in range(ST - MM_LAG, ST):
            emit_MM(jj)
            emit_reduce(jj - 1)
        emit_reduce(ST - 1)
        emit_exp(NG - 1)
        att_drain.clear()
        att_drain.extend([lambda: att_group(NG - 2), lambda: att_group(NG - 1)])
        state[b] = (score_sb, exp_f, exp_t, colsums, att_ps)

    def tail(b, state):
        """Normalize + write outputs for batch row b."""
        _, exp_f, exp_t, colsums, att_ps = state[b]
        # total = sum of colsums: free-dim reduce, then exact fp32 matmul
        # against a ones column for the partition reduction.
        colsum1 = small.tile([P, 1], F32, tag="colsum1", name=f"colsum1_{b}")
        nc.vector.reduce_sum(colsum1, colsums, axis=mybir.AxisListType.X)
        tot_ps = ps_misc.tile([1, 1], F32, tag="m", name=f"tot{b}")
        nc.tensor.matmul(tot_ps, lhsT=colsum1, rhs=ones_colf, start=True, stop=True)
        rcp = small.tile([1, 1], F32, tag="rcp", name=f"rcp{b}")
        nc.vector.reciprocal(rcp, tot_ps)

        # attention = att_ps * (1/total)
        att_sb = small.tile([1, D], F32, tag="attsb", name=f"att_sb{b}")
        nc.vector.tensor_scalar_mul(att_sb, att_ps, rcp)
        nc.sync.dma_start(out=att_out[b:b + 1, :], in_=att_sb)

        # attention_weights = transpose(exp) * (1/total)
        rb_sb = small.tile([ST, 1], F32, tag="rb", name=f"rb{b}")
        nc.gpsimd.partition_broadcast(rb_sb, rcp)
        wt_ps = ps_misc.tile([ST, P], F32, tag="m", name=f"wt_ps{b}")
        nc.tensor.transpose(out=wt_ps, in_=exp_f, identity=ident_f)
        aw_sb = small.tile([ST, P], F32, tag="awsb", name=f"aw_sb{b}")
        nc.vector.tensor_scalar_mul(aw_sb, wt_ps, rb_sb)
        nc.sync.dma_start(out=aw_out[b].rearrange("(j p) -> j p", p=P), in_=aw_sb)

    state = {}
    for b in range(BSH):
        s_loop(b, state)
        if b > 0:
            tail(b - 1, state)
    for fn in att_drain:
        fn()
    att_drain.clear()
    tail(BSH - 1, state)


_NC = None


def _get_nc():
    global _NC
    if _NC is None:
        _NC = build_nc()
    return _NC


def kernel(**inputs):
    query = np.ascontiguousarray(np.asarray(inputs["query"], dtype=np.float32))
    values = np.ascontiguousarray(np.asarray(inputs["values"], dtype=np.float32))
    Ws = np.ascontiguousarray(np.asarray(inputs["Ws"], dtype=np.float32))
    bs = np.ascontiguousarray(np.asarray(inputs["bs"], dtype=np.float32))
    Wt = np.ascontiguousarray(np.asarray(inputs["Wt"], dtype=np.float32))
    bt = np.ascontiguousarray(np.asarray(inputs["bt"], dtype=np.float32))
    Wa = np.ascontiguousarray(np.asarray(inputs["Wa"], dtype=np.float32))
    # `ba` is unused: softmax is invariant to a constant shift of the scores.

    nc = _get_nc()

    from concourse.bass_utils import run_bass_kernel_spmd

    in_maps = []
    for c in range(NCORES):
        sl = slice(c * BSH, (c + 1) * BSH)
        in_maps.append({
            "query": query[sl], "values": values[sl],
            "Ws": Ws, "bs": bs, "Wt": Wt, "bt": bt, "Wa": Wa,
        })
    res = run_bass_kernel_spmd(nc, in_maps, list(range(NCORES))).results

    att = np.concatenate([res[c]["attention"] for c in range(NCORES)], axis=0)
    aw = np.concatenate([res[c]["attention_weights"] for c in range(NCORES)],
                        axis=0)[:, :, None]
    return att.astype(np.float32), aw.astype(np.float32)


# revision 17
# speedup vs baseline: 232.8295x; 232.8295x over previous
"""Trainium2 Bass kernel for Bahdanau-style additive attention.

Computation (per batch row b):
    q_proj = query[b] @ Ws + bs                       # [U]
    v_proj = values[b] @ Wt + bt                      # [S, U]
    score  = tanh(q_proj + v_proj) @ Wa + ba          # [S]
    w      = softmax(score)                           # [S]
    att    = sum_s w[s] * values[b, s]                # [D]
Returns (att [B, D], w [B, S, 1]).

Sharding: data-parallel over batch across 8 NeuronCores (4 rows each);
dense weights replicated.  No collectives needed; results are gathered
on the host.

Notes on the on-device dataflow (per core, per batch row):
  - values arrive in natural [s, d] layout; each [128, 128] block is
    transposed on the PE (transpose-mode matmul) to get the d-major
    layout required as the stationary operand of the projection matmul.
  - projection psum accumulates  valT.T @ Wt  over 4 d-chunks plus a
    K=1 rank-1 matmul that adds (query@Ws + bs + bt) broadcast along
    partitions.
  - tanh on the scalar engine (PSUM -> SBUF), then one DVE
    tensor_tensor_reduce per s-tile computes score = sum_u tanh * Wa.
  - softmax skips the max-subtraction (scores are bounded by ||Wa||_1,
    softmax is shift-invariant; exp stays well inside fp32 range) so
    exp/sums can stream without a batch-global barrier.
  - attention accumulates  exp_col.T @ values_tile  into one psum bank
    over all 32 s-tiles; normalization by 1/sum(exp) happens on the
    [1, 512] result and on the transposed weight tile at the end.
  - matmul operands are viewed as float32r (full-rate fp32 streaming on
    TRN2; plain float32 matmuls run at quarter rate).
"""

import sys

for _p in ("/opt/trn_rl_repo", "/root/.axon_site/_ro/trn_rl_repo"):
    if _p not in sys.path:
        sys.path.append(_p)

from contextlib import ExitStack

import numpy as np

import concourse.bass as bass
import concourse.mybir as mybir
import concourse.tile as tile
from concourse import bacc
from concourse.masks import make_identity

dt = mybir.dt
F32 = dt.float32
F32R = dt.float32r
AF = mybir.ActivationFunctionType
ALU = mybir.AluOpType

B, S, D, U = 32, 4096, 512, 512
NCORES = 8
BSH = B // NCORES  # 4 batch rows per core
P = 128
DC = D // P        # 4 d-chunks
ST = S // P        # 32 s-tiles per batch row
GRP = 4            # s-tiles per group (group = 512 s rows = 1 DMA)
NG = ST // GRP     # 8 groups
ATT_LAG = 2        # groups of lag before emitting attention matmuls
MM_LAG = 2         # s-tiles of lag between transpose stage and projection stage


def _r(ap):
    """View an AP as float32r for PE ops (full-rate fp32 matmul)."""
    return ap.bitcast(F32R)


def ts(i, size):
    return slice(i * size, (i + 1) * size)


def build_nc(repeat=1):
    nc = bacc.Bacc("TRN2", target_bir_lowering=False, debug=False,
                   num_devices=NCORES)

    query = nc.declare_dram_parameter("query", [BSH, D], F32, isOutput=False).ap()
    values = nc.declare_dram_parameter("values", [BSH, S, D], F32, isOutput=False).ap()
    Ws = nc.declare_dram_parameter("Ws", [D, U], F32, isOutput=False).ap()
    bs_d = nc.declare_dram_parameter("bs", [U], F32, isOutput=False).ap()
    Wt = nc.declare_dram_parameter("Wt", [D, U], F32, isOutput=False).ap()
    bt_d = nc.declare_dram_parameter("bt", [U], F32, isOutput=False).ap()
    Wa = nc.declare_dram_parameter("Wa", [U, 1], F32, isOutput=False).ap()
    att_out = nc.declare_dram_parameter("attention", [BSH, D], F32, isOutput=True).ap()
    aw_out = nc.declare_dram_parameter("attention_weights", [BSH, S], F32,
                                       isOutput=True).ap()

    with tile.TileContext(nc) as tc:
        with ExitStack() as ctx:
            _build_program(ctx, tc, query, values, Ws, bs_d, Wt, bt_d, Wa,
                           att_out, aw_out, repeat=repeat)
    nc.compile()
    return nc


def _build_program(ctx, tc, query, values, Ws, bs_d, Wt, bt_d, Wa,
                   att_out, aw_out, repeat=1):
    nc = tc.nc

    const = ctx.enter_context(tc.tile_pool(name="const", bufs=1))
    vals_pool = ctx.enter_context(tc.tile_pool(name="vals", bufs=8))
    vt_pool = ctx.enter_context(tc.tile_pool(name="vt", bufs=5))
    tanh_pool = ctx.enter_context(tc.tile_pool(name="tanh", bufs=3))
    scr_pool = ctx.enter_context(tc.tile_pool(name="scr", bufs=2))
    small = ctx.enter_context(tc.tile_pool(name="small", bufs=2))
    ps_vproj = ctx.enter_context(tc.tile_pool(name="ps_vproj", bufs=3, space="PSUM"))
    ps_valt = ctx.enter_context(tc.tile_pool(name="ps_valt", bufs=2, space="PSUM"))
    ps_att = ctx.enter_context(tc.tile_pool(name="ps_att", bufs=2, space="PSUM"))
    ps_misc = ctx.enter_context(tc.tile_pool(name="ps_misc", bufs=1, space="PSUM"))

    # ---------------- constants ----------------
    ident_f = const.tile([P, P], F32)
    make_identity(nc, ident_f)
    ident = const.tile([P, P], F32R)
    nc.vector.tensor_copy(ident, ident_f)
    ones_f = const.tile([1, P], F32)
    nc.vector.memset(ones_f, 1.0)
    ones_row = const.tile([1, P], F32R)
    nc.vector.tensor_copy(ones_row, ones_f)
    ones_colf = const.tile([P, 1], F32)
    nc.vector.memset(ones_colf, 1.0)

    # query first (tiny, unblocks q-projection), then Ws, then Wt.
    query_sb = const.tile([BSH, D], F32R)
    nc.sync.dma_start(out=query_sb, in_=query.bitcast(F32R))
    ws_sb = const.tile([P, DC, U], F32R)
    nc.sync.dma_start(out=ws_sb, in_=Ws.bitcast(F32R).rearrange("(k p) u -> p k u", p=P))
    wt_sb = const.tile([P, DC, U], F32R)
    nc.sync.dma_start(out=wt_sb, in_=Wt.bitcast(F32R).rearrange("(k p) u -> p k u", p=P))

    # Wa broadcast along partitions: [128, 512], every partition = Wa[:, 0].
    wa_b = const.tile([P, U], F32)
    wa_flat = Wa.rearrange("u one -> (u one)")
    wa_bcast_in = bass.AP(tensor=wa_flat.tensor, offset=wa_flat.offset,
                          ap=[[0, P]] + list(wa_flat.ap))
    nc.sync.dma_start(out=wa_b, in_=wa_bcast_in)

    # bias rows on partition 0
    bs_row = const.tile([1, U], F32)
    nc.sync.dma_start(out=bs_row, in_=bs_d)
    bt_row = const.tile([1, U], F32)
    nc.sync.dma_start(out=bt_row, in_=bt_d)
    bst_row = const.tile([1, U], F32R)
    nc.vector.tensor_add(bst_row, bs_row, bt_row)

    # ---------------- q projection (once per core) ----------------
    qt_sb = const.tile([P, DC, BSH], F32R)  # query transposed, d-chunk k at [:, k, :]
    for k in range(DC):
        qt_ps = ps_valt.tile([P, BSH], F32R, tag="vt_ps", name=f"qt_ps{k}")
        nc.tensor.transpose(out=qt_ps, in_=query_sb[:, ts(k, P)],
                            identity=ident[0:BSH, 0:BSH])
        nc.vector.tensor_copy(qt_sb[:, k, :], qt_ps)

    qproj_ps = ps_vproj.tile([BSH, U], F32, tag="pj", name="qproj_ps")
    for k in range(DC):
        nc.tensor.matmul(qproj_ps, lhsT=qt_sb[:, k, :], rhs=ws_sb[:, k, :],
                         start=(k == 0), stop=False)
    # += (bs + bt) broadcast along partitions via K=1 rank-1 matmul
    nc.tensor.matmul(qproj_ps, lhsT=ones_row[:, 0:BSH], rhs=bst_row,
                     start=False, stop=True)
    qproj_sb = const.tile([BSH, U], F32R)
    nc.vector.tensor_copy(qproj_sb, qproj_ps)

    # per-batch bias rows staged on partition 0 (for the K=1 add matmul)
    qb_stage = []
    for b in range(BSH):
        qb = const.tile([1, U], F32R, name=f"qb_stage{b}")
        nc.sync.dma_start(out=qb, in_=qproj_sb[b:b + 1, :])
        qb_stage.append(qb)

    # ---------------- main pipeline ----------------
    # Emission is software-pipelined so no in-order engine queue convoys:
    #   - the DVE score-reduce for s-tile j is emitted after s-tile j+1's
    #     PE work (its tanh/mul chain lags the PE by ~2 ops),
    #   - exp for group g is emitted one group later,
    #   - attention matmuls lag 2 groups; the last 2 groups drain into the
    #     NEXT batch's loop (att_drain), so the PE never sits on the
    #     exp chain at a batch boundary.
    att_drain = []

    def s_loop(b, state):
        score_sb = small.tile([P, ST], F32, tag="score", name=f"score{b}")
        exp_f = small.tile([P, ST], F32, tag="expf", name=f"expf{b}")
        exp_t = small.tile([P, ST], F32R, tag="exp", name=f"exp{b}")
        colsums = small.tile([P, NG], F32, tag="colsums", name=f"colsums{b}")
        att_ps = ps_att.tile([1, U], F32, tag="att", name=f"att_ps{b}")
        vals_g = []
        prods = {}

        def att_group(g):
            for j in range(GRP):
                jj = g * GRP + j
                nc.tensor.matmul(att_ps, lhsT=exp_t[:, jj:jj + 1],
                                 rhs=vals_g[g][:, j, :],
                                 start=(jj == 0), stop=(jj == ST - 1))

        vts = {}

        def emit_T(g, j):
            """Transpose stage: PE block transposes + psum->sbuf copy."""
            jj = g * GRP + j
            nat = vals_g[g][:, j, :]
            vt_ps = ps_valt.tile([P, D], F32R, tag="vt_ps", name=f"vt_ps{b}_{jj}")
            for k in range(DC):
                nc.tensor.transpose(out=vt_ps[:, ts(k, P)], in_=nat[:, ts(k, P)],
                                    identity=ident)
            vt_sb = vt_pool.tile([P, D], F32R, tag="vt", name=f"vt_sb{b}_{jj}")
            if jj % 2 == 0:
                nc.vector.tensor_copy(vt_sb, vt_ps)
            else:
                nc.scalar.copy(vt_sb, vt_ps)
            vts[jj] = vt_sb

        def emit_MM(jj):
            """Projection stage: 4 accumulating matmuls + bias + tanh + Wa-mul."""
            vt_sb = vts.pop(jj)
            pj = ps_vproj.tile([P, U], F32, tag="pj", name=f"pj{b}_{jj}")
            for k in range(DC):
                nc.tensor.matmul(pj, lhsT=vt_sb[:, ts(k, P)], rhs=wt_sb[:, k, :],
                                 start=(k == 0), stop=False)
            nc.tensor.matmul(pj, lhsT=ones_row, rhs=qb_stage[b], start=False,
                             stop=True)
            th = tanh_pool.tile([P, U], F32, tag="th", name=f"th{b}_{jj}")
            nc.scalar.activation(out=th, in_=pj, func=AF.Tanh)
            prod = scr_pool.tile([P, U], F32, tag="prod", name=f"prod{b}_{jj}")
            nc.gpsimd.tensor_mul(prod, th, wa_b)
            prods[jj] = prod

        def emit_reduce(jj):
            nc.vector.reduce_sum(score_sb[:, jj:jj + 1], prods.pop(jj),
                                 axis=mybir.AxisListType.X)

        def emit_exp(g):
            nc.scalar.activation(out=exp_f[:, ts(g, GRP)],
                                 in_=score_sb[:, ts(g, GRP)],
                                 func=AF.Exp, accum_out=colsums[:, g:g + 1])
            nc.gpsimd.tensor_copy(exp_t[:, ts(g, GRP)], exp_f[:, ts(g, GRP)])

        for g in range(NG):
            vg = vals_pool.tile([P, GRP, D], F32R, tag="vals", name=f"vals{b}_{g}")
            vals_g.append(vg)
            nc.sync.dma_start(
                out=vg,
                in_=values.bitcast(F32R)[b, ts(g, GRP * P), :]
                .rearrange("(t p) d -> p t d", p=P))
            for j in range(GRP):
                jj = g * GRP + j
                emit_T(g, j)
                if jj >= MM_LAG:
                    emit_MM(jj - MM_LAG)
                if jj > MM_LAG:
                    emit_reduce(jj - MM_LAG - 1)
                if j == 3 and g >= 1:
                    emit_exp(g - 1)
                if j == 2:
                    if g >= 2:
                        att_group(g - 2)
                    elif att_drain:
                        att_drain.pop(0)()
        for jj in range(ST - MM_LAG, ST):
            emit_MM(jj)
            emit_reduce(jj - 1)
        emit_reduce(ST - 1)
        emit_exp(NG - 1)
        att_drain.clear()
        att_drain.extend([lambda: att_group(NG - 2), lambda: att_group(NG - 1)])
        state[b] = (score_sb, exp_f, exp_t, colsums, att_ps)

    def tail(b, state):
        """Normalize + write outputs for batch row b."""
        _, exp_f, exp_t, colsums, att_ps = state[b]
        # total = sum of colsums: free-dim reduce, then exact fp32 matmul
        # against a ones column for the partition reduction.
        colsum1 = small.tile([P, 1], F32, tag="colsum1", name=f"colsum1_{b}")
        nc.vector.reduce_sum(colsum1, colsums, axis=mybir.AxisListType.X)
        tot_ps = ps_misc.tile([1, 1], F32, tag="m", name=f"tot{b}")
        nc.tensor.matmul(tot_ps, lhsT=colsum1, rhs=ones_colf, start=True, stop=True)
        rcp = small.tile([1, 1], F32, tag="rcp", name=f"rcp{b}")
        nc.vector.reciprocal(rcp, tot_ps)

        # attention = att_ps * (1/total)
        att_sb = small.tile([1, D], F32, tag="attsb", name=f"att_sb{b}")
        nc.vector.tensor_scalar_mul(att_sb, att_ps, rcp)
        nc.sync.dma_start(out=att_out[b:b + 1, :], in_=att_sb)

        # attention_weights = transpose(exp) * (1/total)
        rb_sb = small.tile([ST, 1], F32, tag="rb", name=f"rb{b}")
        nc.gpsimd.partition_broadcast(rb_sb, rcp)
        wt_ps = ps_misc.tile([ST, P], F32, tag="m", name=f"wt_ps{b}")
        nc.tensor.transpose(out=wt_ps, in_=exp_f, identity=ident_f)
        aw_sb = small.tile([ST, P], F32, tag="awsb", name=f"aw_sb{b}")
        nc.vector.tensor_scalar_mul(aw_sb, wt_ps, rb_sb)
        nc.sync.dma_start(out=aw_out[b].rearrange("(j p) -> j p", p=P), in_=aw_sb)

    def whole_pipeline():
        state = {}
        for b in range(BSH):
            s_loop(b, state)
            if b > 0:
                tail(b - 1, state)
        for fn in att_drain:
            fn()
        att_drain.clear()
        tail(BSH - 1, state)

    if repeat == 1:
        whole_pipeline()
    else:
        with tc.For_i(0, repeat, 1):
            whole_pipeline()


_NC = None


def _get_nc():
    global _NC
    if _NC is None:
        _NC = build_nc()
    return _NC


def kernel(**inputs):
    query = np.ascontiguousarray(np.asarray(inputs["query"], dtype=np.float32))
    values = np.ascontiguousarray(np.asarray(inputs["values"], dtype=np.float32))
    Ws = np.ascontiguousarray(np.asarray(inputs["Ws"], dtype=np.float32))
    bs = np.ascontiguousarray(np.asarray(inputs["bs"], dtype=np.float32))
    Wt = np.ascontiguousarray(np.asarray(inputs["Wt"], dtype=np.float32))
    bt = np.ascontiguousarray(np.asarray(inputs["bt"], dtype=np.float32))
    Wa = np.ascontiguousarray(np.asarray(inputs["Wa"], dtype=np.float32))
    # `ba` is unused: softmax is invariant to a constant shift of the scores.

    nc = _get_nc()

    from concourse.bass_utils import run_bass_kernel_spmd

    in_maps = []
    for c in range(NCORES):
        sl = slice(c * BSH, (c + 1) * BSH)
        in_maps.append({
            "query": query[sl], "values": values[sl],
            "Ws": Ws, "bs": bs, "Wt": Wt, "bt": bt, "Wa": Wa,
        })
    res = run_bass_kernel_spmd(nc, in_maps, list(range(NCORES))).results

    att = np.concatenate([res[c]["attention"] for c in range(NCORES)], axis=0)
    aw = np.concatenate([res[c]["attention_weights"] for c in range(NCORES)],
                        axis=0)[:, :, None]
    return att.astype(np.float32), aw.astype(np.float32)


# revision 18
# speedup vs baseline: 276.1081x; 1.1859x over previous
"""Trainium2 Bass kernel for Bahdanau-style additive attention.

Computation (per batch row b):
    q_proj = query[b] @ Ws + bs                       # [U]
    v_proj = values[b] @ Wt + bt                      # [S, U]
    score  = tanh(q_proj + v_proj) @ Wa + ba          # [S]
    w      = softmax(score)                           # [S]
    att    = sum_s w[s] * values[b, s]                # [D]
Returns (att [B, D], w [B, S, 1]).

Sharding: data-parallel over batch across 8 NeuronCores (4 rows each);
dense weights replicated.  No collectives needed; results are gathered
on the host.

Notes on the on-device dataflow (per core, per batch row):
  - values arrive in natural [s, d] layout; each [128, 128] block is
    transposed on the PE (transpose-mode matmul) to get the d-major
    layout required as the stationary operand of the projection matmul.
  - projection psum accumulates  valT.T @ Wt  over 4 d-chunks plus a
    K=1 rank-1 matmul that adds (query@Ws + bs + bt) broadcast along
    partitions.
  - tanh on the scalar engine (PSUM -> SBUF), then one DVE
    tensor_tensor_reduce per s-tile computes score = sum_u tanh * Wa.
  - softmax skips the max-subtraction (scores are bounded by ||Wa||_1,
    softmax is shift-invariant; exp stays well inside fp32 range) so
    exp/sums can stream without a batch-global barrier.
  - attention accumulates  exp_col.T @ values_tile  into one psum bank
    over all 32 s-tiles; normalization by 1/sum(exp) happens on the
    [1, 512] result and on the transposed weight tile at the end.
  - matmul operands are viewed as float32r (full-rate fp32 streaming on
    TRN2; plain float32 matmuls run at quarter rate).
"""

import sys

for _p in ("/opt/trn_rl_repo", "/root/.axon_site/_ro/trn_rl_repo"):
    if _p not in sys.path:
        sys.path.append(_p)

from contextlib import ExitStack

import numpy as np

import concourse.bass as bass
import concourse.mybir as mybir
import concourse.tile as tile
from concourse import bacc
from concourse.masks import make_identity

dt = mybir.dt
F32 = dt.float32
F32R = dt.float32r
AF = mybir.ActivationFunctionType
ALU = mybir.AluOpType

B, S, D, U = 32, 4096, 512, 512
NCORES = 8
BSH = B // NCORES  # 4 batch rows per core
P = 128
DC = D // P        # 4 d-chunks
ST = S // P        # 32 s-tiles per batch row
GRP = 4            # s-tiles per group (group = 512 s rows = 1 DMA)
NG = ST // GRP     # 8 groups
ATT_LAG = 2        # groups of lag before emitting attention matmuls
MM_LAG = 2         # s-tiles of lag between transpose stage and projection stage


def _r(ap):
    """View an AP as float32r for PE ops (full-rate fp32 matmul)."""
    return ap.bitcast(F32R)


def ts(i, size):
    return slice(i * size, (i + 1) * size)


def build_nc(repeat=1):
    nc = bacc.Bacc("TRN2", target_bir_lowering=False, debug=False,
                   num_devices=NCORES)

    query = nc.declare_dram_parameter("query", [BSH, D], F32, isOutput=False).ap()
    values = nc.declare_dram_parameter("values", [BSH, S, D], F32, isOutput=False).ap()
    Ws = nc.declare_dram_parameter("Ws", [D, U], F32, isOutput=False).ap()
    bs_d = nc.declare_dram_parameter("bs", [U], F32, isOutput=False).ap()
    Wt = nc.declare_dram_parameter("Wt", [D, U], F32, isOutput=False).ap()
    bt_d = nc.declare_dram_parameter("bt", [U], F32, isOutput=False).ap()
    Wa = nc.declare_dram_parameter("Wa", [U, 1], F32, isOutput=False).ap()
    att_out = nc.declare_dram_parameter("attention", [BSH, D], F32, isOutput=True).ap()
    aw_out = nc.declare_dram_parameter("attention_weights", [BSH, S], F32,
                                       isOutput=True).ap()

    with tile.TileContext(nc) as tc:
        with ExitStack() as ctx:
            _build_program(ctx, tc, query, values, Ws, bs_d, Wt, bt_d, Wa,
                           att_out, aw_out, repeat=repeat)
    nc.compile()
    return nc


def _build_program(ctx, tc, query, values, Ws, bs_d, Wt, bt_d, Wa,
                   att_out, aw_out, repeat=1):
    nc = tc.nc

    const = ctx.enter_context(tc.tile_pool(name="const", bufs=1))
    vals_pool = ctx.enter_context(tc.tile_pool(name="vals", bufs=8))
    vt_pool = ctx.enter_context(tc.tile_pool(name="vt", bufs=5))
    tanh_pool = ctx.enter_context(tc.tile_pool(name="tanh", bufs=3))
    scr_pool = ctx.enter_context(tc.tile_pool(name="scr", bufs=2))
    small = ctx.enter_context(tc.tile_pool(name="small", bufs=2))
    ps_vproj = ctx.enter_context(tc.tile_pool(name="ps_vproj", bufs=3, space="PSUM"))
    ps_valt = ctx.enter_context(tc.tile_pool(name="ps_valt", bufs=2, space="PSUM"))
    ps_att = ctx.enter_context(tc.tile_pool(name="ps_att", bufs=2, space="PSUM"))
    ps_misc = ctx.enter_context(tc.tile_pool(name="ps_misc", bufs=1, space="PSUM"))

    # ---------------- constants ----------------
    ident_f = const.tile([P, P], F32)
    make_identity(nc, ident_f)
    ident = const.tile([P, P], F32R)
    nc.vector.tensor_copy(ident, ident_f)
    ones_f = const.tile([1, P], F32)
    nc.vector.memset(ones_f, 1.0)
    ones_row = const.tile([1, P], F32R)
    nc.vector.tensor_copy(ones_row, ones_f)
    ones_colf = const.tile([P, 1], F32)
    nc.vector.memset(ones_colf, 1.0)

    # query first (tiny, unblocks q-projection), then Ws, then Wt.
    query_sb = const.tile([BSH, D], F32R)
    nc.sync.dma_start(out=query_sb, in_=query.bitcast(F32R))
    ws_sb = const.tile([P, DC, U], F32R)
    nc.sync.dma_start(out=ws_sb, in_=Ws.bitcast(F32R).rearrange("(k p) u -> p k u", p=P))
    wt_sb = const.tile([P, DC, U], F32R)
    nc.sync.dma_start(out=wt_sb, in_=Wt.bitcast(F32R).rearrange("(k p) u -> p k u", p=P))

    # Wa broadcast along partitions: [128, 512], every partition = Wa[:, 0].
    wa_b = const.tile([P, U], F32)
    wa_flat = Wa.rearrange("u one -> (u one)")
    wa_bcast_in = bass.AP(tensor=wa_flat.tensor, offset=wa_flat.offset,
                          ap=[[0, P]] + list(wa_flat.ap))
    nc.sync.dma_start(out=wa_b, in_=wa_bcast_in)

    # bias rows on partition 0
    bs_row = const.tile([1, U], F32)
    nc.sync.dma_start(out=bs_row, in_=bs_d)
    bt_row = const.tile([1, U], F32)
    nc.sync.dma_start(out=bt_row, in_=bt_d)
    bst_row = const.tile([1, U], F32R)
    nc.vector.tensor_add(bst_row, bs_row, bt_row)

    # ---------------- q projection (once per core) ----------------
    qt_sb = const.tile([P, DC, BSH], F32R)  # query transposed, d-chunk k at [:, k, :]
    for k in range(DC):
        qt_ps = ps_valt.tile([P, BSH], F32R, tag="vt_ps", name=f"qt_ps{k}")
        nc.tensor.transpose(out=qt_ps, in_=query_sb[:, ts(k, P)],
                            identity=ident[0:BSH, 0:BSH])
        nc.vector.tensor_copy(qt_sb[:, k, :], qt_ps)

    qproj_ps = ps_vproj.tile([BSH, U], F32, tag="pj", name="qproj_ps")
    for k in range(DC):
        nc.tensor.matmul(qproj_ps, lhsT=qt_sb[:, k, :], rhs=ws_sb[:, k, :],
                         start=(k == 0), stop=False)
    # += (bs + bt) broadcast along partitions via K=1 rank-1 matmul
    nc.tensor.matmul(qproj_ps, lhsT=ones_row[:, 0:BSH], rhs=bst_row,
                     start=False, stop=True)
    qproj_sb = const.tile([BSH, U], F32R)
    nc.vector.tensor_copy(qproj_sb, qproj_ps)

    # per-batch bias rows staged on partition 0 (for the K=1 add matmul)
    qb_stage = []
    for b in range(BSH):
        qb = const.tile([1, U], F32R, name=f"qb_stage{b}")
        nc.sync.dma_start(out=qb, in_=qproj_sb[b:b + 1, :])
        qb_stage.append(qb)

    # ---------------- main pipeline ----------------
    # Emission is software-pipelined so no in-order engine queue convoys:
    #   - the DVE score-reduce for s-tile j is emitted after s-tile j+1's
    #     PE work (its tanh/mul chain lags the PE by ~2 ops),
    #   - exp for group g is emitted one group later,
    #   - attention matmuls lag 2 groups; the last 2 groups drain into the
    #     NEXT batch's loop (att_drain), so the PE never sits on the
    #     exp chain at a batch boundary.
    att_drain = []

    def s_loop(b, state):
        score_sb = small.tile([P, ST], F32, tag="score", name=f"score{b}")
        exp_f = small.tile([P, ST], F32, tag="expf", name=f"expf{b}")
        exp_t = small.tile([P, ST], F32R, tag="exp", name=f"exp{b}")
        colsums = small.tile([P, NG], F32, tag="colsums", name=f"colsums{b}")
        att_ps = ps_att.tile([1, U], F32, tag="att", name=f"att_ps{b}")
        qb_bcast = small.tile([P, U], F32, tag="qbb", name=f"qbb{b}")
        nc.gpsimd.partition_broadcast(qb_bcast, qb_stage[b].bitcast(F32))
        vals_g = []
        prods = {}

        def att_group(g):
            for j in range(GRP):
                jj = g * GRP + j
                nc.tensor.matmul(att_ps, lhsT=exp_t[:, jj:jj + 1],
                                 rhs=vals_g[g][:, j, :],
                                 start=(jj == 0), stop=(jj == ST - 1))

        vts = {}

        def emit_T(g, j):
            """Transpose stage: PE block transposes + psum->sbuf copy."""
            jj = g * GRP + j
            nat = vals_g[g][:, j, :]
            vt_ps = ps_valt.tile([P, D], F32R, tag="vt_ps", name=f"vt_ps{b}_{jj}")
            for k in range(DC):
                nc.tensor.transpose(out=vt_ps[:, ts(k, P)], in_=nat[:, ts(k, P)],
                                    identity=ident)
            vt_sb = vt_pool.tile([P, D], F32R, tag="vt", name=f"vt_sb{b}_{jj}")
            if jj % 2 == 0:
                nc.vector.tensor_copy(vt_sb, vt_ps)
            else:
                nc.scalar.copy(vt_sb, vt_ps)
            vts[jj] = vt_sb

        def emit_MM(jj):
            """Projection stage: 4 accumulating matmuls + bias + tanh + Wa-mul."""
            vt_sb = vts.pop(jj)
            pj = ps_vproj.tile([P, U], F32, tag="pj", name=f"pj{b}_{jj}")
            for k in range(DC):
                nc.tensor.matmul(pj, lhsT=vt_sb[:, ts(k, P)], rhs=wt_sb[:, k, :],
                                 start=(k == 0), stop=(k == DC - 1))
            arg = tanh_pool.tile([P, U], F32, tag="arg", name=f"arg{b}_{jj}")
            nc.vector.tensor_add(arg, pj, qb_bcast)
            th = tanh_pool.tile([P, U], F32, tag="th", name=f"th{b}_{jj}")
            nc.scalar.activation(out=th, in_=arg, func=AF.Tanh)
            prod = scr_pool.tile([P, U], F32, tag="prod", name=f"prod{b}_{jj}")
            nc.gpsimd.tensor_mul(prod, th, wa_b)
            prods[jj] = prod

        def emit_reduce(jj):
            nc.vector.reduce_sum(score_sb[:, jj:jj + 1], prods.pop(jj),
                                 axis=mybir.AxisListType.X)

        def emit_exp(g):
            nc.scalar.activation(out=exp_f[:, ts(g, GRP)],
                                 in_=score_sb[:, ts(g, GRP)],
                                 func=AF.Exp, accum_out=colsums[:, g:g + 1])
            nc.gpsimd.tensor_copy(exp_t[:, ts(g, GRP)], exp_f[:, ts(g, GRP)])

        for g in range(NG):
            vg = vals_pool.tile([P, GRP, D], F32R, tag="vals", name=f"vals{b}_{g}")
            vals_g.append(vg)
            nc.sync.dma_start(
                out=vg,
                in_=values.bitcast(F32R)[b, ts(g, GRP * P), :]
                .rearrange("(t p) d -> p t d", p=P))
            for j in range(GRP):
                jj = g * GRP + j
                emit_T(g, j)
                if jj >= MM_LAG:
                    emit_MM(jj - MM_LAG)
                if jj > MM_LAG:
                    emit_reduce(jj - MM_LAG - 1)
                if j == 3 and g >= 1:
                    emit_exp(g - 1)
                if j == 2:
                    if g >= 2:
                        att_group(g - 2)
                    elif att_drain:
                        att_drain.pop(0)()
        for jj in range(ST - MM_LAG, ST):
            emit_MM(jj)
            emit_reduce(jj - 1)
        emit_reduce(ST - 1)
        emit_exp(NG - 1)
        att_drain.clear()
        att_drain.extend([lambda: att_group(NG - 2), lambda: att_group(NG - 1)])
        state[b] = (score_sb, exp_f, exp_t, colsums, att_ps)

    def tail(b, state):
        """Normalize + write outputs for batch row b."""
        _, exp_f, exp_t, colsums, att_ps = state[b]
        # total = sum of colsums: free-dim reduce, then exact fp32 matmul
        # against a ones column for the partition reduction.
        colsum1 = small.tile([P, 1], F32, tag="colsum1", name=f"colsum1_{b}")
        nc.vector.reduce_sum(colsum1, colsums, axis=mybir.AxisListType.X)
        tot_ps = ps_misc.tile([1, 1], F32, tag="m", name=f"tot{b}")
        nc.tensor.matmul(tot_ps, lhsT=colsum1, rhs=ones_colf, start=True, stop=True)
        rcp = small.tile([1, 1], F32, tag="rcp", name=f"rcp{b}")
        nc.vector.reciprocal(rcp, tot_ps)

        # attention = att_ps * (1/total)
        att_sb = small.tile([1, D], F32, tag="attsb", name=f"att_sb{b}")
        nc.vector.tensor_scalar_mul(att_sb, att_ps, rcp)
        nc.sync.dma_start(out=att_out[b:b + 1, :], in_=att_sb)

        # attention_weights = transpose(exp) * (1/total)
        rb_sb = small.tile([ST, 1], F32, tag="rb", name=f"rb{b}")
        nc.gpsimd.partition_broadcast(rb_sb, rcp)
        wt_ps = ps_misc.tile([ST, P], F32, tag="m", name=f"wt_ps{b}")
        nc.tensor.transpose(out=wt_ps, in_=exp_f, identity=ident_f)
        aw_sb = small.tile([ST, P], F32, tag="awsb", name=f"aw_sb{b}")
        nc.vector.tensor_scalar_mul(aw_sb, wt_ps, rb_sb)
        nc.sync.dma_start(out=aw_out[b].rearrange("(j p) -> j p", p=P), in_=aw_sb)

    def whole_pipeline():
        state = {}
        for b in range(BSH):
            s_loop(b, state)
            if b > 0:
                tail(b - 1, state)
        for fn in att_drain:
            fn()
        att_drain.clear()
        tail(BSH - 1, state)

    if repeat == 1:
        whole_pipeline()
    else:
        with tc.For_i(0, repeat, 1):
            whole_pipeline()


_NC = None


def _get_nc():
    global _NC
    if _NC is None:
        _NC = build_nc()
    return _NC


def kernel(**inputs):
    query = np.ascontiguousarray(np.asarray(inputs["query"], dtype=np.float32))
    values = np.ascontiguousarray(np.asarray(inputs["values"], dtype=np.float32))
    Ws = np.ascontiguousarray(np.asarray(inputs["Ws"], dtype=np.float32))
    bs = np.ascontiguousarray(np.asarray(inputs["bs"], dtype=np.float32))
    Wt = np.ascontiguousarray(np.asarray(inputs["Wt"], dtype=np.float32))
    bt = np.ascontiguousarray(np.asarray(inputs["bt"], dtype=np.float32))
    Wa = np.ascontiguousarray(np.asarray(inputs["Wa"], dtype=np.float32))
    # `ba` is unused: softmax is invariant to a constant shift of the scores.

    nc = _get_nc()

    from concourse.bass_utils import run_bass_kernel_spmd

    in_maps = []
    for c in range(NCORES):
        sl = slice(c * BSH, (c + 1) * BSH)
        in_maps.append({
            "query": query[sl], "values": values[sl],
            "Ws": Ws, "bs": bs, "Wt": Wt, "bt": bt, "Wa": Wa,
        })
    res = run_bass_kernel_spmd(nc, in_maps, list(range(NCORES))).results

    att = np.concatenate([res[c]["attention"] for c in range(NCORES)], axis=0)
    aw = np.concatenate([res[c]["attention_weights"] for c in range(NCORES)],
                        axis=0)[:, :, None]
    return att.astype(np.float32), aw.astype(np.float32)


# revision 19
# speedup vs baseline: 281.3748x; 1.0191x over previous
"""Trainium2 Bass kernel for Bahdanau-style additive attention.

Computation (per batch row b):
    q_proj = query[b] @ Ws + bs                       # [U]
    v_proj = values[b] @ Wt + bt                      # [S, U]
    score  = tanh(q_proj + v_proj) @ Wa + ba          # [S]
    w      = softmax(score)                           # [S]
    att    = sum_s w[s] * values[b, s]                # [D]
Returns (att [B, D], w [B, S, 1]).

Sharding: data-parallel over batch across 8 NeuronCores (4 rows each);
dense weights replicated.  No collectives needed; results are gathered
on the host.

Notes on the on-device dataflow (per core, per batch row):
  - values arrive in natural [s, d] layout; each [128, 128] block is
    transposed on the PE (transpose-mode matmul) to get the d-major
    layout required as the stationary operand of the projection matmul.
  - projection psum accumulates  valT.T @ Wt  over 4 d-chunks plus a
    K=1 rank-1 matmul that adds (query@Ws + bs + bt) broadcast along
    partitions.
  - tanh on the scalar engine (PSUM -> SBUF), then one DVE
    tensor_tensor_reduce per s-tile computes score = sum_u tanh * Wa.
  - softmax skips the max-subtraction (scores are bounded by ||Wa||_1,
    softmax is shift-invariant; exp stays well inside fp32 range) so
    exp/sums can stream without a batch-global barrier.
  - attention accumulates  exp_col.T @ values_tile  into one psum bank
    over all 32 s-tiles; normalization by 1/sum(exp) happens on the
    [1, 512] result and on the transposed weight tile at the end.
  - matmul operands are viewed as float32r (full-rate fp32 streaming on
    TRN2; plain float32 matmuls run at quarter rate).
"""

import sys

for _p in ("/opt/trn_rl_repo", "/root/.axon_site/_ro/trn_rl_repo"):
    if _p not in sys.path:
        sys.path.append(_p)

from contextlib import ExitStack

import numpy as np

import concourse.bass as bass
import concourse.mybir as mybir
import concourse.tile as tile
from concourse import bacc
from concourse.masks import make_identity

dt = mybir.dt
F32 = dt.float32
F32R = dt.float32r
BF16 = dt.bfloat16
AF = mybir.ActivationFunctionType
ALU = mybir.AluOpType

B, S, D, U = 32, 4096, 512, 512
NCORES = 8
BSH = B // NCORES  # 4 batch rows per core
P = 128
DC = D // P        # 4 d-chunks
ST = S // P        # 32 s-tiles per batch row
GRP = 4            # s-tiles per group (group = 512 s rows = 1 DMA)
NG = ST // GRP     # 8 groups
ATT_LAG = 2        # groups of lag before emitting attention matmuls
MM_LAG = 2         # s-tiles of lag between transpose stage and projection stage


def _r(ap):
    """View an AP as float32r for PE ops (full-rate fp32 matmul)."""
    return ap.bitcast(F32R)


def ts(i, size):
    return slice(i * size, (i + 1) * size)


def build_nc(repeat=1):
    nc = bacc.Bacc("TRN2", target_bir_lowering=False, debug=False,
                   num_devices=NCORES)

    query = nc.declare_dram_parameter("query", [BSH, D], F32, isOutput=False).ap()
    values = nc.declare_dram_parameter("values", [BSH, S, D], F32, isOutput=False).ap()
    Ws = nc.declare_dram_parameter("Ws", [D, U], F32, isOutput=False).ap()
    bs_d = nc.declare_dram_parameter("bs", [U], F32, isOutput=False).ap()
    Wt = nc.declare_dram_parameter("Wt", [D, U], F32, isOutput=False).ap()
    bt_d = nc.declare_dram_parameter("bt", [U], F32, isOutput=False).ap()
    Wa = nc.declare_dram_parameter("Wa", [U, 1], F32, isOutput=False).ap()
    att_out = nc.declare_dram_parameter("attention", [BSH, D], F32, isOutput=True).ap()
    aw_out = nc.declare_dram_parameter("attention_weights", [BSH, S], F32,
                                       isOutput=True).ap()

    with tile.TileContext(nc) as tc:
        with ExitStack() as ctx:
            _build_program(ctx, tc, query, values, Ws, bs_d, Wt, bt_d, Wa,
                           att_out, aw_out, repeat=repeat)
    nc.compile()
    return nc


def _build_program(ctx, tc, query, values, Ws, bs_d, Wt, bt_d, Wa,
                   att_out, aw_out, repeat=1):
    nc = tc.nc

    const = ctx.enter_context(tc.tile_pool(name="const", bufs=1))
    vals_pool = ctx.enter_context(tc.tile_pool(name="vals", bufs=8))
    vt_pool = ctx.enter_context(tc.tile_pool(name="vt", bufs=5))
    tanh_pool = ctx.enter_context(tc.tile_pool(name="tanh", bufs=3))
    scr_pool = ctx.enter_context(tc.tile_pool(name="scr", bufs=2))
    small = ctx.enter_context(tc.tile_pool(name="small", bufs=2))
    ps_vproj = ctx.enter_context(tc.tile_pool(name="ps_vproj", bufs=3, space="PSUM"))
    ps_valt = ctx.enter_context(tc.tile_pool(name="ps_valt", bufs=2, space="PSUM"))
    ps_att = ctx.enter_context(tc.tile_pool(name="ps_att", bufs=2, space="PSUM"))
    ps_misc = ctx.enter_context(tc.tile_pool(name="ps_misc", bufs=1, space="PSUM"))

    # ---------------- constants ----------------
    ident_f = const.tile([P, P], F32)
    make_identity(nc, ident_f)
    ident = const.tile([P, P], F32R)
    nc.vector.tensor_copy(ident, ident_f)
    ones_f = const.tile([1, P], F32)
    nc.vector.memset(ones_f, 1.0)
    ones_row = const.tile([1, P], F32R)
    nc.vector.tensor_copy(ones_row, ones_f)
    ones_colf = const.tile([P, 1], F32)
    nc.vector.memset(ones_colf, 1.0)

    # query first (tiny, unblocks q-projection), then Ws, then Wt.
    query_sb = const.tile([BSH, D], F32R)
    nc.sync.dma_start(out=query_sb, in_=query.bitcast(F32R))
    ws_sb = const.tile([P, DC, U], F32R)
    nc.sync.dma_start(out=ws_sb, in_=Ws.bitcast(F32R).rearrange("(k p) u -> p k u", p=P))
    wt_f32 = const.tile([P, DC, U], F32)
    nc.sync.dma_start(out=wt_f32, in_=Wt.rearrange("(k p) u -> p k u", p=P))
    wt_sb = const.tile([P, DC, U], BF16)
    nc.vector.tensor_copy(wt_sb, wt_f32)

    # Wa broadcast along partitions: [128, 512], every partition = Wa[:, 0].
    wa_b = const.tile([P, U], F32)
    wa_flat = Wa.rearrange("u one -> (u one)")
    wa_bcast_in = bass.AP(tensor=wa_flat.tensor, offset=wa_flat.offset,
                          ap=[[0, P]] + list(wa_flat.ap))
    nc.sync.dma_start(out=wa_b, in_=wa_bcast_in)

    # bias rows on partition 0
    bs_row = const.tile([1, U], F32)
    nc.sync.dma_start(out=bs_row, in_=bs_d)
    bt_row = const.tile([1, U], F32)
    nc.sync.dma_start(out=bt_row, in_=bt_d)
    bst_row = const.tile([1, U], F32R)
    nc.vector.tensor_add(bst_row, bs_row, bt_row)

    # ---------------- q projection (once per core) ----------------
    qt_sb = const.tile([P, DC, BSH], F32R)  # query transposed, d-chunk k at [:, k, :]
    for k in range(DC):
        qt_ps = ps_valt.tile([P, BSH], F32R, tag="vt_ps", name=f"qt_ps{k}")
        nc.tensor.transpose(out=qt_ps, in_=query_sb[:, ts(k, P)],
                            identity=ident[0:BSH, 0:BSH])
        nc.vector.tensor_copy(qt_sb[:, k, :], qt_ps)

    qproj_ps = ps_vproj.tile([BSH, U], F32, tag="pj", name="qproj_ps")
    for k in range(DC):
        nc.tensor.matmul(qproj_ps, lhsT=qt_sb[:, k, :], rhs=ws_sb[:, k, :],
                         start=(k == 0), stop=False)
    # += (bs + bt) broadcast along partitions via K=1 rank-1 matmul
    nc.tensor.matmul(qproj_ps, lhsT=ones_row[:, 0:BSH], rhs=bst_row,
                     start=False, stop=True)
    qproj_sb = const.tile([BSH, U], F32R)
    nc.vector.tensor_copy(qproj_sb, qproj_ps)

    # per-batch bias rows staged on partition 0 (for the K=1 add matmul)
    qb_stage = []
    for b in range(BSH):
        qb = const.tile([1, U], F32R, name=f"qb_stage{b}")
        nc.sync.dma_start(out=qb, in_=qproj_sb[b:b + 1, :])
        qb_stage.append(qb)

    # ---------------- main pipeline ----------------
    # Emission is software-pipelined so no in-order engine queue convoys:
    #   - the DVE score-reduce for s-tile j is emitted after s-tile j+1's
    #     PE work (its tanh/mul chain lags the PE by ~2 ops),
    #   - exp for group g is emitted one group later,
    #   - attention matmuls lag 2 groups; the last 2 groups drain into the
    #     NEXT batch's loop (att_drain), so the PE never sits on the
    #     exp chain at a batch boundary.
    att_drain = []

    def s_loop(b, state):
        score_sb = small.tile([P, ST], F32, tag="score", name=f"score{b}")
        exp_f = small.tile([P, ST], F32, tag="expf", name=f"expf{b}")
        exp_t = small.tile([P, ST], F32R, tag="exp", name=f"exp{b}")
        colsums = small.tile([P, NG], F32, tag="colsums", name=f"colsums{b}")
        att_ps = ps_att.tile([1, U], F32, tag="att", name=f"att_ps{b}")
        qb_bcast = small.tile([P, U], F32, tag="qbb", name=f"qbb{b}")
        nc.gpsimd.partition_broadcast(qb_bcast, qb_stage[b].bitcast(F32))
        vals_g = []
        prods = {}

        def att_group(g):
            for j in range(GRP):
                jj = g * GRP + j
                nc.tensor.matmul(att_ps, lhsT=exp_t[:, jj:jj + 1],
                                 rhs=vals_g[g][:, j, :],
                                 start=(jj == 0), stop=(jj == ST - 1))

        vts = {}

        def emit_T(g, j):
            """Transpose stage: PE block transposes + psum->sbuf copy."""
            jj = g * GRP + j
            nat = vals_g[g][:, j, :]
            vt_ps = ps_valt.tile([P, D], F32R, tag="vt_ps", name=f"vt_ps{b}_{jj}")
            for k in range(DC):
                nc.tensor.transpose(out=vt_ps[:, ts(k, P)], in_=nat[:, ts(k, P)],
                                    identity=ident)
            vt_sb = vt_pool.tile([P, D], BF16, tag="vt", name=f"vt_sb{b}_{jj}")
            if jj % 2 == 0:
                nc.vector.tensor_copy(vt_sb, vt_ps)
            else:
                nc.scalar.copy(vt_sb, vt_ps)
            vts[jj] = vt_sb

        def emit_MM(jj):
            """Projection stage: 4 accumulating matmuls + bias + tanh + Wa-mul."""
            vt_sb = vts.pop(jj)
            pj = ps_vproj.tile([P, U], F32, tag="pj", name=f"pj{b}_{jj}")
            for k in range(DC):
                nc.tensor.matmul(pj, lhsT=vt_sb[:, ts(k, P)], rhs=wt_sb[:, k, :],
                                 start=(k == 0), stop=(k == DC - 1))
            arg = tanh_pool.tile([P, U], F32, tag="arg", name=f"arg{b}_{jj}")
            nc.vector.tensor_add(arg, pj, qb_bcast)
            th = tanh_pool.tile([P, U], F32, tag="th", name=f"th{b}_{jj}")
            nc.scalar.activation(out=th, in_=arg, func=AF.Tanh)
            prod = scr_pool.tile([P, U], F32, tag="prod", name=f"prod{b}_{jj}")
            nc.gpsimd.tensor_mul(prod, th, wa_b)
            prods[jj] = prod

        def emit_reduce(jj):
            nc.vector.reduce_sum(score_sb[:, jj:jj + 1], prods.pop(jj),
                                 axis=mybir.AxisListType.X)

        def emit_exp(g):
            nc.scalar.activation(out=exp_f[:, ts(g, GRP)],
                                 in_=score_sb[:, ts(g, GRP)],
                                 func=AF.Exp, accum_out=colsums[:, g:g + 1])
            nc.gpsimd.tensor_copy(exp_t[:, ts(g, GRP)], exp_f[:, ts(g, GRP)])

        for g in range(NG):
            vg = vals_pool.tile([P, GRP, D], F32R, tag="vals", name=f"vals{b}_{g}")
            vals_g.append(vg)
            nc.sync.dma_start(
                out=vg,
                in_=values.bitcast(F32R)[b, ts(g, GRP * P), :]
                .rearrange("(t p) d -> p t d", p=P))
            for j in range(GRP):
                jj = g * GRP + j
                emit_T(g, j)
                if jj >= MM_LAG:
                    emit_MM(jj - MM_LAG)
                if jj > MM_LAG:
                    emit_reduce(jj - MM_LAG - 1)
                if j == 3 and g >= 1:
                    emit_exp(g - 1)
                if j == 2:
                    if g >= 2:
                        att_group(g - 2)
                    elif att_drain:
                        att_drain.pop(0)()
        for jj in range(ST - MM_LAG, ST):
            emit_MM(jj)
            emit_reduce(jj - 1)
        emit_reduce(ST - 1)
        emit_exp(NG - 1)
        att_drain.clear()
        att_drain.extend([lambda: att_group(NG - 2), lambda: att_group(NG - 1)])
        state[b] = (score_sb, exp_f, exp_t, colsums, att_ps)

    def tail(b, state):
        """Normalize + write outputs for batch row b."""
        _, exp_f, exp_t, colsums, att_ps = state[b]
        # total = sum of colsums: free-dim reduce, then exact fp32 matmul
        # against a ones column for the partition reduction.
        colsum1 = small.tile([P, 1], F32, tag="colsum1", name=f"colsum1_{b}")
        nc.vector.reduce_sum(colsum1, colsums, axis=mybir.AxisListType.X)
        tot_ps = ps_misc.tile([1, 1], F32, tag="m", name=f"tot{b}")
        nc.tensor.matmul(tot_ps, lhsT=colsum1, rhs=ones_colf, start=True, stop=True)
        rcp = small.tile([1, 1], F32, tag="rcp", name=f"rcp{b}")
        nc.vector.reciprocal(rcp, tot_ps)

        # attention = att_ps * (1/total)
        att_sb = small.tile([1, D], F32, tag="attsb", name=f"att_sb{b}")
        nc.vector.tensor_scalar_mul(att_sb, att_ps, rcp)
        nc.sync.dma_start(out=att_out[b:b + 1, :], in_=att_sb)

        # attention_weights = transpose(exp) * (1/total)
        rb_sb = small.tile([ST, 1], F32, tag="rb", name=f"rb{b}")
        nc.gpsimd.partition_broadcast(rb_sb, rcp)
        wt_ps = ps_misc.tile([ST, P], F32, tag="m", name=f"wt_ps{b}")
        nc.tensor.transpose(out=wt_ps, in_=exp_f, identity=ident_f)
        aw_sb = small.tile([ST, P], F32, tag="awsb", name=f"aw_sb{b}")
        nc.vector.tensor_scalar_mul(aw_sb, wt_ps, rb_sb)
        nc.sync.dma_start(out=aw_out[b].rearrange("(j p) -> j p", p=P), in_=aw_sb)

    def whole_pipeline():
        state = {}
        for b in range(BSH):
            s_loop(b, state)
            if b > 0:
                tail(b - 1, state)
        for fn in att_drain:
            fn()
        att_drain.clear()
        tail(BSH - 1, state)

    if repeat == 1:
        whole_pipeline()
    else:
        with tc.For_i(0, repeat, 1):
            whole_pipeline()


_NC = None


def _get_nc():
    global _NC
    if _NC is None:
        _NC = build_nc()
    return _NC


def kernel(**inputs):
    query = np.ascontiguousarray(np.asarray(inputs["query"], dtype=np.float32))
    values = np.ascontiguousarray(np.asarray(inputs["values"], dtype=np.float32))
    Ws = np.ascontiguousarray(np.asarray(inputs["Ws"], dtype=np.float32))
    bs = np.ascontiguousarray(np.asarray(inputs["bs"], dtype=np.float32))
    Wt = np.ascontiguousarray(np.asarray(inputs["Wt"], dtype=np.float32))
    bt = np.ascontiguousarray(np.asarray(inputs["bt"], dtype=np.float32))
    Wa = np.ascontiguousarray(np.asarray(inputs["Wa"], dtype=np.float32))
    # `ba` is unused: softmax is invariant to a constant shift of the scores.

    nc = _get_nc()

    from concourse.bass_utils import run_bass_kernel_spmd

    in_maps = []
    for c in range(NCORES):
        sl = slice(c * BSH, (c + 1) * BSH)
        in_maps.append({
            "query": query[sl], "values": values[sl],
            "Ws": Ws, "bs": bs, "Wt": Wt, "bt": bt, "Wa": Wa,
        })
    res = run_bass_kernel_spmd(nc, in_maps, list(range(NCORES))).results

    att = np.concatenate([res[c]["attention"] for c in range(NCORES)], axis=0)
    aw = np.concatenate([res[c]["attention_weights"] for c in range(NCORES)],
                        axis=0)[:, :, None]
    return att.astype(np.float32), aw.astype(np.float32)
